# revision 27
# baseline (speedup 1.0000x reference)
"""Trainium2 Bass kernel for DualHazardHead (moe_routing).

Computation per token t:
  x = concat(h, a, d, age)            [594]
  z = gelu(x @ Wt + bt)               [256]
  pw = softmax(h @ Wr + br)           [7]
  inst  = z @ Wbi + bbi + sum_p pw_p (z @ Wei_p + bei_p)   [20]
  group = z @ Wbg + bbg + sum_p pw_p (z @ Weg_p + beg_p)   [20]

Sharding: pure data parallel over B (32 -> 4 per core) on 8 NeuronCores.

v3 design (per core, NTOK=8192 tokens, 16 macro tiles of 512):
  - x transposed feature-major ON HOST: xT[m, k, b, j] bf16, zero-padded
    594 -> 640 with a constant-1.0 row at feature 608 (k=96 of block 4).
  - trunk zT [256, tok] via 2x5 accumulating matmuls; trunk bias rides
    the ones row (wt[96, 4, :] = trunk_b); exact GELU on ACT -> zs bf16.
    pz0 chain runs first so GELU0 overlaps the pz1 chain.
  - router: logits for macro m+1 are computed DURING macro m (16 tiny
    token-major matmuls over h blocks 0-3 only).  Router bias is folded
    into the softmax as a per-phase scale invC = exp(-router_b) on the
    tanh-identity exp: e_p = (t+1)/((t-1)*invC_p) = -exp(l_p+b_p); all
    signs cancel in pw = e/S, and slot 7 (=S) normalizes to exactly 1.0
    for the base head.
  - head biases: pw8 (token-major, bf16, padded [128,4x32]) is PE-
    transposed to pwT (one 128-cycle bf16 transpose), copied to SBUF by
    ACT, then a K=8 matmul per subtile (lhsT = pwT slots, rhs = bias
    table) accumulates sum_p pw_p*be_p + bb into pe4 cols 320:360 --
    replacing v2's 4x320-col ones-row bias matmuls (~8.5us PE saved).
  - heads: z matmuls (2 blocks x 4 subtiles) + bias-dot per bank; ACT
    evacuates pe4 PSUM -> SBUF bf16 in subtile pairs; DVE multiply by
    pw8 runs in 2x mode (all-bf16 SBUF), DVE reduce -> f32; Pool adds
    the bias-dot columns (bf16 out) and issues the output DMA.
  - PSUM: pz0+pz1 (2 banks) + pe4 (4) + ppw router bank (1, even/odd
    32-col halves) + pwT bank (1, even/odd 128-col halves) = 8 exactly.
  - 8 dependency-free warm-up matmuls at t=0 keep the PE's HAM activity
    window open during initial transfers (starts at 2.4 GHz not 1.2).
"""

import numpy as np

B, T = 32, 2048
HID, ACTD, SRC, AGE = 512, 64, 2, 16
TRUNK, BINS, PHASES = 256, 20, 7
IN_DIM = HID + ACTD + SRC + AGE  # 594
NCORES = 8
B_LOC = B // NCORES  # 4
NTOK = B_LOC * T  # 8192
MACRO = 512
NMACRO = NTOK // MACRO  # 16
SUB = MACRO // 128  # 4
NBLK = 5  # 594 features zero-padded to 5 k-blocks of 128
NHK = 2 * BINS  # 40 (head, bin) pairs
NP8 = PHASES + 1  # 7 experts + 1 base slot
NCOL = NHK * NP8  # 320 z-matmul output columns
BCOL = NHK  # 40 bias-dot columns (pe4 cols NCOL:NCOL+BCOL)

NWARM = 8
_BUILT = {}
LAST_RESULT = None


def _build_module():
    """Build the Bass module (same NEFF for all cores)."""
    import concourse.bass as bass
    import concourse.tile as tile
    from concourse import bacc, mybir

    f32 = mybir.dt.float32
    bf16 = mybir.dt.bfloat16
    AF = mybir.ActivationFunctionType
    ALU = mybir.AluOpType
    ts = bass.ts

    nc = bacc.Bacc("TRN2", target_bir_lowering=False, debug=False)

    x_d = nc.dram_tensor("x", [NMACRO, 128, NBLK, MACRO], bf16, kind="ExternalInput")
    wt_d = nc.dram_tensor("wt", [128, NBLK, TRUNK], bf16, kind="ExternalInput")
    wr_d = nc.dram_tensor("wr", [128, 4, PHASES], bf16, kind="ExternalInput")
    wh_d = nc.dram_tensor("wh", [128, 2, NCOL], bf16, kind="ExternalInput")
    bt_d = nc.dram_tensor("bt", [128, 2, BCOL], bf16, kind="ExternalInput")
    ic_d = nc.dram_tensor("invc", [128, PHASES], f32, kind="ExternalInput")
    id_d = nc.dram_tensor("ident", [128, 128], bf16, kind="ExternalInput")
    out_d = nc.dram_tensor("out", [NTOK, NHK], bf16, kind="ExternalOutput")

    ov = out_d[:, :].rearrange("(m s p) hk -> m p s hk", p=128, s=SUB)

    with tile.TileContext(nc) as tc:
        with (
            tc.tile_pool(name="const", bufs=1) as const,
            tc.tile_pool(name="xin", bufs=3) as xin,
            tc.tile_pool(name="zs", bufs=2) as zsp,
            tc.tile_pool(name="sm", bufs=3) as smp,
            tc.tile_pool(name="pwts", bufs=2) as pwts,
            tc.tile_pool(name="evac", bufs=2) as evacp,
            tc.tile_pool(name="prod", bufs=2) as prodp,
            tc.tile_pool(name="outp", bufs=3) as outp,
            tc.tile_pool(name="ps_z", bufs=2, space="PSUM") as ps_z,
            tc.tile_pool(name="ps_e", bufs=1, space="PSUM") as ps_e,
            tc.tile_pool(name="ps_m", bufs=1, space="PSUM") as ps_m,
            tc.tile_pool(name="ps_t", bufs=1, space="PSUM") as ps_t,
        ):
            wt = const.tile([128, NBLK, TRUNK], bf16)
            nc.gpsimd.dma_start(wt[:, 0, :], wt_d[:, 0, :])
            nc.gpsimd.dma_start(wt[:, 1:5, :], wt_d[:, 1:5, :])
            wr = const.tile([128, 4, PHASES], bf16)
            nc.scalar.dma_start(wr, wr_d[:])
            wh = const.tile([128, 2, NCOL], bf16)
            nc.scalar.dma_start(wh, wh_d[:])
            bt = const.tile([128, 2, BCOL], bf16)
            nc.scalar.dma_start(bt, bt_d[:])
            ident = const.tile([128, 128], bf16)
            nc.scalar.dma_start(ident, id_d[:])
            invc = const.tile([128, PHASES], f32)
            nc.scalar.dma_start(invc, ic_d[:])

            # persistent PSUM scratch: router-logit bank (even/odd 32-col
            # halves) and a separate pwT transpose bank (sharing one bank
            # creates false tile-level WAR deps between ACT and PE)
            ppw = ps_m.tile([128, 512], f32)
            pwt_ps = ps_t.tile([128, 128], bf16)

            pe4 = ps_e.tile([128, SUB, 512], f32, tag="pe4")
            # HAM warm-up: dummy matmuls with no DMA dependency keep the
            # PE busy through its 3.4us activity window during the initial
            # transfers, so the real work starts at 2.4 GHz.
            wu = const.tile([128, MACRO], bf16)
            nc.vector.memset(wu, 0.0)
            # force ONE combined ACT table load (gelu+tanh+copy) during
            # the startup DMA window instead of a mid-stream reload
            wuf = const.tile([128, 8], f32)
            nc.scalar.activation(out=wuf, in_=wu[:, 0:8], func=AF.Gelu)
            nc.scalar.activation(out=wuf, in_=wu[:, 0:8], func=AF.Tanh)
            nc.scalar.copy(out=wuf, in_=wu[:, 0:8])
            for _ in range(NWARM):
                nc.tensor.matmul(
                    pe4[:, 0, 0:MACRO], wu[:, 0:128], wu, start=True, stop=True
                )
            # PE prewarm: consume each const via a dummy matmul so later
            # real PE instructions never need a startup semaphore wait.
            nc.tensor.matmul(
                pe4[:, 0, 0:128], wt[:, 0, 0:128], wt[:, 0, 0:128],
                start=True, stop=True,
            )
            nc.tensor.matmul(
                pe4[:PHASES, 0, 0:128], wr[:, 0, :], wt[:, 0, 0:128],
                start=True, stop=True,
            )
            nc.tensor.matmul(
                pe4[:, 1, 0:NCOL], wh[:, 0, 0:128], wh[:, 1, :],
                start=True, stop=True,
            )
            nc.tensor.matmul(
                pe4[:, 2, 0:BCOL], ident, bt[:, 0, :], start=True, stop=True,
            )

            def emit_router(mi, xtt):
                """16 token-major router matmuls for macro mi (h blocks
                0-3 only; groups sequential within the shared ppw bank)."""
                off = 32 * (mi % 2)
                for s in range(SUB):
                    c0 = off + s * 8
                    for rb in range(4):
                        nc.tensor.matmul(
                            ppw[:, c0 : c0 + PHASES],
                            xtt[:, rb, ts(s, 128)], wr[:, rb, :],
                            start=(rb == 0), stop=(rb == 3),
                        )

            def emit_softmax(mi):
                """tanh-identity softmax for macro mi -> pw8 bf16 padded
                [128, SUB*32] with slot7 == 1.0 (base head weight)."""
                off = 32 * (mi % 2)
                lg = ppw[:, off : off + 32].rearrange("p (s q) -> p s q", q=8)
                th = smp.tile([128, SUB, PHASES], f32, tag="th")
                nc.scalar.activation(
                    out=th, in_=lg[:, :, 0:PHASES], func=AF.Tanh, scale=0.5,
                )
                den = smp.tile([128, SUB, PHASES], f32, tag="den")
                # den = (t - 1) * invC  (negative; signs cancel in pw)
                nc.vector.scalar_tensor_tensor(
                    out=den, in0=th, scalar=1.0,
                    in1=invc[:, None, :].to_broadcast([128, SUB, PHASES]),
                    op0=ALU.subtract, op1=ALU.mult,
                )
                nc.vector.reciprocal_approx_fast(out=den, in_=den)
                e8 = smp.tile([128, SUB, NP8], f32, tag="e8")
                # e_p = (t + 1) * den  (= -exp(l_p + b_p) up to scale)
                nc.vector.scalar_tensor_tensor(
                    out=e8[:, :, 0:PHASES], in0=th, scalar=1.0, in1=den,
                    op0=ALU.add, op1=ALU.mult,
                )
                nc.vector.reduce_sum(
                    out=e8[:, :, PHASES], in_=e8[:, :, 0:PHASES],
                    axis=mybir.AxisListType.X,
                )
                recS = smp.tile([128, SUB], f32, tag="recS")
                nc.vector.reciprocal_approx_fast(out=recS, in_=e8[:, :, PHASES])
                pw8 = smp.tile([128, SUB, 32], bf16, tag="pw8")
                # zero the padding slots (on Pool) so the transposed
                # garbage can never be NaN when it multiplies the zero
                # rows of the masked bias tables
                nc.gpsimd.memset(pw8[:, :, NP8:32], 0.0)
                nc.vector.tensor_tensor(
                    out=pw8[:, :, 0:NP8], in0=e8,
                    in1=recS[:, :, None].to_broadcast([128, SUB, NP8]),
                    op=ALU.mult,
                )
                return pw8

            def emit_transpose(mi, pw8):
                """PE-transpose pw8 -> pwT [s*32+slot, tok]; ACT copies
                PSUM -> SBUF bf16 for use as bias-dot lhsT."""
                nc.tensor.transpose(
                    pwt_ps, pw8[:, :, :].rearrange("p s q -> p (s q)"), ident,
                )
                pwt_sb = pwts.tile([128, 128], bf16, tag="pwt")
                nc.scalar.copy(out=pwt_sb, in_=pwt_ps)
                return pwt_sb

            pw8_cur = None
            pwt_cur = None
            xt_cur = xin.tile([128, NBLK, MACRO], bf16)
            # per-block loads so the very first trunk matmul only waits
            # for 128 KB
            for b in range(3):
                nc.sync.dma_start(xt_cur[:, b, :], x_d[0, :, b, :])
            nc.sync.dma_start(xt_cur[:, 3:5, :], x_d[0, :, 3:5, :])

            for m in range(NMACRO):
                xt = xt_cur
                if m + 1 < NMACRO:
                    xt_next = xin.tile([128, NBLK, MACRO], bf16)
                    nc.sync.dma_start(xt_next[:, 0:3, :], x_d[m + 1, :, 0:3, :])
                    nc.sync.dma_start(xt_next[:, 3:5, :], x_d[m + 1, :, 3:5, :])
                    xt_cur = xt_next

                # ping-pong: swap which buffer is pz0/pz1 each macro, so
                # trunk(m+1)'s first chain WARs against GELU1(m) (always
                # prompt: z-blk1(m) already needed it) instead of GELU0.
                ta = ps_z.tile([128, MACRO], f32, tag="pz")
                tb = ps_z.tile([128, MACRO], f32, tag="pz")
                pz0, pz1 = (ta, tb) if m % 2 == 0 else (tb, ta)
                zs = zsp.tile([128, 2, MACRO], bf16)

                if m == 0:
                    # col-interleaved so each x block is consumed as it
                    # lands; router(0) woven in once block 3 is here.
                    for b in range(3):
                        for pz, c in ((pz0, 0), (pz1, 128)):
                            nc.tensor.matmul(
                                pz, wt[:, b, c : c + 128], xt[:, b, :],
                                start=(b == 0), stop=False,
                            )
                    nc.tensor.matmul(
                        pz0, wt[:, 3, 0:128], xt[:, 3, :], start=False, stop=False
                    )
                    nc.tensor.matmul(
                        pz0, wt[:, 4, 0:128], xt[:, 4, :], start=False, stop=True
                    )
                    nc.scalar.activation(
                        out=zs[:, 0, :], in_=pz0, func=AF.Gelu, scale=1.0
                    )
                    emit_router(0, xt)
                    # softmax(0) emitted before GELU1 so tanh(0) runs on
                    # ACT as soon as the router matmuls finish
                    pw8_cur = emit_softmax(0)
                    nc.tensor.matmul(
                        pz1, wt[:, 3, 128:256], xt[:, 3, :], start=False, stop=False
                    )
                    nc.tensor.matmul(
                        pz1, wt[:, 4, 128:256], xt[:, 4, :], start=False, stop=True
                    )
                    nc.scalar.activation(
                        out=zs[:, 1, :], in_=pz1, func=AF.Gelu, scale=1.0
                    )
                else:
                    # pz0 chain first so GELU0 overlaps the pz1 chain;
                    # pwT transpose for this macro woven in early.
                    for b in range(NBLK):
                        nc.tensor.matmul(
                            pz0, wt[:, b, 0:128], xt[:, b, :],
                            start=(b == 0), stop=(b == 4),
                        )
                        if b == 1:
                            pwt_cur = emit_transpose(m, pw8_cur)
                    nc.scalar.activation(
                        out=zs[:, 0, :], in_=pz0, func=AF.Gelu, scale=1.0
                    )
                    for b in range(NBLK):
                        nc.tensor.matmul(
                            pz1, wt[:, b, 128:256], xt[:, b, :],
                            start=(b == 0), stop=(b == 4),
                        )
                    nc.scalar.activation(
                        out=zs[:, 1, :], in_=pz1, func=AF.Gelu, scale=1.0
                    )

                pe4 = ps_e.tile([128, SUB, 512], f32, tag="pe4")
                # z block 0 opens each bank's accumulation group
                for s in range(SUB):
                    nc.tensor.matmul(
                        pe4[:, s, 0:NCOL], zs[:, 0, ts(s, 128)], wh[:, 0, :],
                        start=True, stop=False,
                    )
                # router + softmax for the NEXT macro run here so pw8 is
                # always ready a full macro before its bias-dot.
                if m + 1 < NMACRO:
                    emit_router(m + 1, xt_cur)
                    pw8_next = emit_softmax(m + 1)
                else:
                    pw8_next = None
                if m == 0:
                    pwt_cur = emit_transpose(0, pw8_cur)
                for s in range(SUB):
                    nc.tensor.matmul(
                        pe4[:, s, 0:NCOL], zs[:, 1, ts(s, 128)], wh[:, 1, :],
                        start=False, stop=False,
                    )
                # bias-dot: out[tok, hk] = sum_p pw_p * be_p[hk] + bb[hk]
                pe4s = evacp.tile([128, SUB, BCOL], bf16)
                for s in range(SUB):
                    # K=64 from base {0,64}: subtile s slots live at
                    # partitions 32s..32s+8; the masked bias table zeroes
                    # every other contraction row.
                    q = 64 * (s // 2)
                    nc.tensor.matmul(
                        pe4[:, s, NCOL : NCOL + BCOL],
                        pwt_cur[q : q + 64, :],
                        bt[q : q + 64, s % 2, :],
                        start=False, stop=True, skip_group_check=True,
                    )
                # evac AFTER all bias-dots (an ACT read of pe4 between PE
                # writes to it serializes the PE queue): only the 40
                # bias-dot cols leave PSUM via ACT; the big products read
                # PSUM directly on DVE.
                nc.scalar.copy(
                    out=pe4s, in_=pe4[:, :, NCOL : NCOL + BCOL],
                )

                # combine: prod = pe4 * pw8 (DVE, PSUM read) -> bf16;
                # 2-level p-reduction: bf16 pair-add at DVE 2x, then f32
                # reduce of 4; Pool adds the bias-dot columns.
                osb = outp.tile([128, SUB, NHK], f32, tag="osb")
                prod = prodp.tile([128, SUB, NHK, NP8], bf16, tag="prod")
                h4 = prodp.tile([128, SUB, NHK, 4], bf16, tag="h4")
                for p2 in range(2):
                    sl = slice(2 * p2, 2 * p2 + 2)
                    nc.vector.tensor_tensor(
                        out=prod[:, sl],
                        in0=pe4[:, sl, 0:NCOL].rearrange(
                            "p s (hk e) -> p s hk e", e=NP8
                        ),
                        in1=pw8_cur[:, sl, None, 0:NP8].to_broadcast(
                            [128, 2, NHK, NP8]
                        ),
                        op=ALU.mult,
                    )
                    nc.vector.tensor_tensor(
                        out=h4[:, sl], in0=prod[:, sl, :, 0:4],
                        in1=prod[:, sl, :, 4:8], op=ALU.add,
                    )
                    nc.vector.reduce_sum(
                        out=osb[:, sl], in_=h4[:, sl],
                        axis=mybir.AxisListType.X,
                    )
                osb2 = outp.tile([128, SUB, NHK], bf16, tag="osb2")
                nc.gpsimd.tensor_tensor(
                    out=osb2, in0=osb, in1=pe4s, op=ALU.add,
                )
                # output DMA on the same queue that produced osb2: FIFO
                # order means the issue never blocks another queue
                nc.gpsimd.dma_start(ov[m], osb2)

                pw8_cur = pw8_next

    nc.compile()
    return nc


def _host_weights(inp):
    """Rearrange weights into on-device layouts (host-side, one-time)."""
    import ml_dtypes

    bf16 = ml_dtypes.bfloat16
    f = np.float32

    # trunk weights; block 4 row 96 multiplies the ones row -> trunk bias
    wt = np.zeros((128, NBLK, TRUNK), f)
    for b in range(4):
        wt[:, b, :] = inp["trunk_w"][b * 128 : (b + 1) * 128]
    wt[:82, 4, :] = inp["trunk_w"][512:IN_DIM]
    wt[96, 4, :] = inp["trunk_b"]

    wr = np.zeros((128, 4, PHASES), f)
    for b in range(4):
        wr[:, b, :] = inp["router_w"][b * 128 : (b + 1) * 128]

    # z-matmul weights: col c = (h*20+k)*8 + p; p<7 experts, p=7 base
    wh_full = np.zeros((TRUNK, NHK, NP8), f)
    wh_full[:, :BINS, :PHASES] = np.transpose(inp["inst_exp_w"], (1, 2, 0))
    wh_full[:, BINS:, :PHASES] = np.transpose(inp["group_exp_w"], (1, 2, 0))
    wh_full[:, :BINS, PHASES] = inp["inst_base_w"]
    wh_full[:, BINS:, PHASES] = inp["group_base_w"]
    wh = (
        wh_full.reshape(TRUNK, NCOL).reshape(2, 128, NCOL).transpose(1, 0, 2)
    ).astype(f)

    # masked bias tables for the K=64 bias-dot: table s%2 has dr at
    # partitions 32s..32s+8 and zeros elsewhere
    dr = np.zeros((NP8, NHK), f)
    dr[:PHASES, :BINS] = inp["inst_exp_b"]
    dr[:PHASES, BINS:] = inp["group_exp_b"]
    dr[PHASES, :BINS] = inp["inst_base_b"]
    dr[PHASES, BINS:] = inp["group_base_b"]
    bt = np.zeros((128, 2, BCOL), f)
    for s in range(4):
        bt[32 * s : 32 * s + NP8, s % 2, :] = dr

    invc = np.tile(np.exp(-inp["router_b"])[None, :], (128, 1)).astype(f)
    ident = np.eye(128, dtype=f)

    return (
        wt.astype(bf16), wr.astype(bf16), wh.astype(bf16), bt.astype(bf16),
        invc, ident.astype(bf16),
    )


def kernel(**inputs):
    global LAST_RESULT
    import sys

    if "/opt/trn_rl_repo" not in sys.path:
        sys.path.insert(0, "/opt/trn_rl_repo")
    import ml_dtypes
    from concourse.bass_utils import run_bass_kernel_spmd

    bf16 = ml_dtypes.bfloat16

    inp = {k: np.asarray(v, dtype=np.float32) for k, v in inputs.items()}

    if "nc" not in _BUILT:
        _BUILT["nc"] = _build_module()
    nc = _BUILT["nc"]

    wt, wr, wh, bt, invc, ident = _host_weights(inp)

    x_full = np.concatenate(
        [inp["h_t"], inp["a_t"], inp["d_t"], inp["age_embed"]], axis=-1
    ).astype(bf16)  # [B, T, 594]
    # feature-major transpose + zero-pad 594 -> 640 (5 blocks of 128):
    # xT[c, m, k, b, j] = x[c, m*512 + j, b*128 + k]; padded feature 608
    # (k=96 of block 4) is a constant 1.0 that carries the trunk bias.
    pad = np.zeros((NCORES, NMACRO, MACRO, NBLK * 128 - IN_DIM), bf16)
    pad[:, :, :, 608 - IN_DIM] = 1.0
    x_pad = np.concatenate(
        [x_full.reshape(NCORES, NMACRO, MACRO, IN_DIM), pad], axis=-1
    ).reshape(NCORES, NMACRO, MACRO, NBLK, 128)
    xT = np.ascontiguousarray(np.transpose(x_pad, (0, 1, 4, 3, 2)))

    in_maps = []
    for c in range(NCORES):
        in_maps.append(
            {
                "x": xT[c], "wt": wt, "wr": wr, "wh": wh, "bt": bt,
                "invc": invc, "ident": ident,
            }
        )

    res = run_bass_kernel_spmd(nc, in_maps, core_ids=list(range(NCORES)))
    LAST_RESULT = res

    inst = np.empty((B, T, BINS), np.float32)
    grp = np.empty((B, T, BINS), np.float32)
    for c in range(NCORES):
        o = res.results[c]["out"].astype(np.float32)
        inst[c * B_LOC : (c + 1) * B_LOC] = o[:, 0:BINS].reshape(B_LOC, T, BINS)
        grp[c * B_LOC : (c + 1) * B_LOC] = o[:, BINS:].reshape(B_LOC, T, BINS)
    return inst, grp


# revision 32
# speedup vs baseline: 1.2184x; 1.2184x over previous
"""Trainium2 Bass kernel for DualHazardHead (moe_routing).

Computation per token t:
  x = concat(h, a, d, age)            [594]
  z = gelu(x @ Wt + bt)               [256]
  pw = softmax(h @ Wr + br)           [7]
  inst  = z @ Wbi + bbi + sum_p pw_p (z @ Wei_p + bei_p)   [20]
  group = z @ Wbg + bbg + sum_p pw_p (z @ Weg_p + beg_p)   [20]

Sharding: pure data parallel over B (32 -> 4 per core) on 8 NeuronCores.

v3 design (per core, NTOK=8192 tokens, 16 macro tiles of 512):
  - x transposed feature-major ON HOST: xT[m, k, b, j] bf16, zero-padded
    594 -> 640 with a constant-1.0 row at feature 608 (k=96 of block 4).
  - trunk zT [256, tok] via 2x5 accumulating matmuls; trunk bias rides
    the ones row (wt[96, 4, :] = trunk_b); exact GELU on ACT -> zs bf16.
    pz0 chain runs first so GELU0 overlaps the pz1 chain.
  - router: logits for macro m+1 are computed DURING macro m (16 tiny
    token-major matmuls over h blocks 0-3 only).  Router bias is folded
    into the softmax as a per-phase scale invC = exp(-router_b) on the
    tanh-identity exp: e_p = (t+1)/((t-1)*invC_p) = -exp(l_p+b_p); all
    signs cancel in pw = e/S, and slot 7 (=S) normalizes to exactly 1.0
    for the base head.
  - head biases: pw8 (token-major, bf16, padded [128,4x32]) is PE-
    transposed to pwT (one 128-cycle bf16 transpose), copied to SBUF by
    ACT, then a K=8 matmul per subtile (lhsT = pwT slots, rhs = bias
    table) accumulates sum_p pw_p*be_p + bb into pe4 cols 320:360 --
    replacing v2's 4x320-col ones-row bias matmuls (~8.5us PE saved).
  - heads: z matmuls (2 blocks x 4 subtiles) + bias-dot per bank; ACT
    evacuates pe4 PSUM -> SBUF bf16 in subtile pairs; DVE multiply by
    pw8 runs in 2x mode (all-bf16 SBUF), DVE reduce -> f32; Pool adds
    the bias-dot columns (bf16 out) and issues the output DMA.
  - PSUM: pz0+pz1 (2 banks) + pe4 (4) + ppw router bank (1, even/odd
    32-col halves) + pwT bank (1, even/odd 128-col halves) = 8 exactly.
  - 8 dependency-free warm-up matmuls at t=0 keep the PE's HAM activity
    window open during initial transfers (starts at 2.4 GHz not 1.2).
"""

import numpy as np

B, T = 32, 2048
HID, ACTD, SRC, AGE = 512, 64, 2, 16
TRUNK, BINS, PHASES = 256, 20, 7
IN_DIM = HID + ACTD + SRC + AGE  # 594
NCORES = 8
B_LOC = B // NCORES  # 4
NTOK = B_LOC * T  # 8192
MACRO = 512
NMACRO = NTOK // MACRO  # 16
SUB = MACRO // 128  # 4
NBLK = 5  # 594 features zero-padded to 5 k-blocks of 128
NHK = 2 * BINS  # 40 (head, bin) pairs
NP8 = PHASES + 1  # 7 experts + 1 base slot
NCOL = NHK * NP8  # 320 z-matmul output columns
BCOL = NHK  # 40 bias-dot columns (pe4 cols NCOL:NCOL+BCOL)

NWARM = 8
_BUILT = {}
LAST_RESULT = None


def _build_module():
    """Build the Bass module (same NEFF for all cores)."""
    import concourse.bass as bass
    import concourse.tile as tile
    from concourse import bacc, mybir

    f32 = mybir.dt.float32
    bf16 = mybir.dt.bfloat16
    AF = mybir.ActivationFunctionType
    ALU = mybir.AluOpType
    ts = bass.ts

    nc = bacc.Bacc("TRN2", target_bir_lowering=False, debug=False)

    x_d = nc.dram_tensor("x", [NMACRO, 128, NBLK, MACRO], bf16, kind="ExternalInput")
    wt_d = nc.dram_tensor("wt", [128, NBLK, TRUNK], bf16, kind="ExternalInput")
    wr_d = nc.dram_tensor("wr", [128, 4, PHASES], bf16, kind="ExternalInput")
    wh_d = nc.dram_tensor("wh", [128, 2, NCOL], bf16, kind="ExternalInput")
    bt_d = nc.dram_tensor("bt", [128, 2, BCOL], bf16, kind="ExternalInput")
    ic_d = nc.dram_tensor("invc", [128, PHASES], f32, kind="ExternalInput")
    id_d = nc.dram_tensor("ident", [128, 128], bf16, kind="ExternalInput")
    out_d = nc.dram_tensor("out", [NTOK, NHK], bf16, kind="ExternalOutput")

    ov = out_d[:, :].rearrange("(m s p) hk -> m p s hk", p=128, s=SUB)

    with tile.TileContext(nc) as tc:
        with (
            tc.tile_pool(name="const", bufs=1) as const,
            tc.tile_pool(name="xin", bufs=3) as xin,
            tc.tile_pool(name="zs", bufs=2) as zsp,
            tc.tile_pool(name="sm", bufs=3) as smp,
            tc.tile_pool(name="pwts", bufs=2) as pwts,
            tc.tile_pool(name="evac", bufs=2) as evacp,
            tc.tile_pool(name="prod", bufs=2) as prodp,
            tc.tile_pool(name="outp", bufs=3) as outp,
            tc.tile_pool(name="ps_z", bufs=2, space="PSUM") as ps_z,
            tc.tile_pool(name="ps_e", bufs=1, space="PSUM") as ps_e,
            tc.tile_pool(name="ps_m", bufs=1, space="PSUM") as ps_m,
            tc.tile_pool(name="ps_t", bufs=1, space="PSUM") as ps_t,
        ):
            wt = const.tile([128, NBLK, TRUNK], bf16)
            nc.gpsimd.dma_start(wt[:, 0, :], wt_d[:, 0, :])
            nc.gpsimd.dma_start(wt[:, 1:5, :], wt_d[:, 1:5, :])
            wr = const.tile([128, 4, PHASES], bf16)
            nc.scalar.dma_start(wr, wr_d[:])
            wh = const.tile([128, 2, NCOL], bf16)
            nc.scalar.dma_start(wh, wh_d[:])
            bt = const.tile([128, 2, BCOL], bf16)
            nc.scalar.dma_start(bt, bt_d[:])
            ident = const.tile([128, 128], bf16)
            nc.scalar.dma_start(ident, id_d[:])
            invc = const.tile([128, PHASES], f32)
            nc.scalar.dma_start(invc, ic_d[:])

            # persistent PSUM scratch: router-logit bank (even/odd 32-col
            # halves) and a separate pwT transpose bank (sharing one bank
            # creates false tile-level WAR deps between ACT and PE)
            ppw = ps_m.tile([128, 512], f32)
            pwt_ps = ps_t.tile([128, 128], bf16)

            pe4 = ps_e.tile([128, SUB, 512], f32, tag="pe4")
            # HAM warm-up: dummy matmuls with no DMA dependency keep the
            # PE busy through its 3.4us activity window during the initial
            # transfers, so the real work starts at 2.4 GHz.
            wu = const.tile([128, MACRO], bf16)
            nc.vector.memset(wu, 0.0)
            # force ONE combined ACT table load (gelu+tanh+copy) during
            # the startup DMA window instead of a mid-stream reload
            wuf = const.tile([128, 8], f32)
            nc.scalar.activation(out=wuf, in_=wu[:, 0:8], func=AF.Gelu)
            nc.scalar.activation(out=wuf, in_=wu[:, 0:8], func=AF.Tanh)
            nc.scalar.copy(out=wuf, in_=wu[:, 0:8])
            for _ in range(NWARM):
                nc.tensor.matmul(
                    pe4[:, 0, 0:MACRO], wu[:, 0:128], wu, start=True, stop=True
                )
            # PE prewarm: consume each const via a dummy matmul so later
            # real PE instructions never need a startup semaphore wait.
            nc.tensor.matmul(
                pe4[:, 0, 0:128], wt[:, 0, 0:128], wt[:, 0, 0:128],
                start=True, stop=True,
            )
            nc.tensor.matmul(
                pe4[:PHASES, 0, 0:128], wr[:, 0, :], wt[:, 0, 0:128],
                start=True, stop=True,
            )
            nc.tensor.matmul(
                pe4[:, 1, 0:NCOL], wh[:, 0, 0:128], wh[:, 1, :],
                start=True, stop=True,
            )
            nc.tensor.matmul(
                pe4[:, 2, 0:BCOL], ident, bt[:, 0, :], start=True, stop=True,
            )

            def emit_router(mi, xtt):
                """16 token-major router matmuls for macro mi (h blocks
                0-3 only; groups sequential within the shared ppw bank)."""
                off = 32 * (mi % 2)
                for s in range(SUB):
                    c0 = off + s * 8
                    for rb in range(4):
                        nc.tensor.matmul(
                            ppw[:, c0 : c0 + PHASES],
                            xtt[:, rb, ts(s, 128)], wr[:, rb, :],
                            start=(rb == 0), stop=(rb == 3),
                        )

            def emit_softmax(mi):
                """tanh-identity softmax for macro mi -> pw8 bf16 padded
                [128, SUB*32] with slot7 == 1.0 (base head weight)."""
                off = 32 * (mi % 2)
                lg = ppw[:, off : off + 32].rearrange("p (s q) -> p s q", q=8)
                th = smp.tile([128, SUB, PHASES], f32, tag="th")
                nc.scalar.activation(
                    out=th, in_=lg[:, :, 0:PHASES], func=AF.Tanh, scale=0.5,
                )
                den = smp.tile([128, SUB, PHASES], f32, tag="den")
                # den = (t - 1) * invC  (negative; signs cancel in pw)
                nc.vector.scalar_tensor_tensor(
                    out=den, in0=th, scalar=1.0,
                    in1=invc[:, None, :].to_broadcast([128, SUB, PHASES]),
                    op0=ALU.subtract, op1=ALU.mult,
                )
                nc.vector.reciprocal_approx_fast(out=den, in_=den)
                e8 = smp.tile([128, SUB, NP8], f32, tag="e8")
                # e_p = (t + 1) * den  (= -exp(l_p + b_p) up to scale)
                nc.vector.scalar_tensor_tensor(
                    out=e8[:, :, 0:PHASES], in0=th, scalar=1.0, in1=den,
                    op0=ALU.add, op1=ALU.mult,
                )
                nc.vector.reduce_sum(
                    out=e8[:, :, PHASES], in_=e8[:, :, 0:PHASES],
                    axis=mybir.AxisListType.X,
                )
                recS = smp.tile([128, SUB], f32, tag="recS")
                nc.vector.reciprocal_approx_fast(out=recS, in_=e8[:, :, PHASES])
                pw8 = smp.tile([128, SUB, 32], bf16, tag="pw8")
                # zero the padding slots so the transposed garbage can
                # never be NaN when it multiplies the zero rows of the
                # masked bias tables; only the first pass per rotation
                # buffer sees uninitialized SBUF, later passes see stale
                # finite pw values.
                if mi < 3:
                    nc.gpsimd.memset(pw8[:, :, NP8:32], 0.0)
                nc.vector.tensor_tensor(
                    out=pw8[:, :, 0:NP8], in0=e8,
                    in1=recS[:, :, None].to_broadcast([128, SUB, NP8]),
                    op=ALU.mult,
                )
                return pw8

            def emit_transpose(mi, pw8):
                """PE-transpose pw8 -> pwT [s*32+slot, tok]; ACT copies
                PSUM -> SBUF bf16 for use as bias-dot lhsT."""
                nc.tensor.transpose(
                    pwt_ps, pw8[:, :, :].rearrange("p s q -> p (s q)"), ident,
                )
                pwt_sb = pwts.tile([128, 128], bf16, tag="pwt")
                # copy on DVE (bf16 2x) to keep the ACT queue short
                nc.vector.tensor_copy(out=pwt_sb, in_=pwt_ps)
                return pwt_sb

            pw8_cur = None
            pwt_cur = None
            xt_cur = xin.tile([128, NBLK, MACRO], bf16)
            # per-block loads so the very first trunk matmul only waits
            # for 128 KB
            for b in range(3):
                nc.sync.dma_start(xt_cur[:, b, :], x_d[0, :, b, :])
            nc.sync.dma_start(xt_cur[:, 3:5, :], x_d[0, :, 3:5, :])

            for m in range(NMACRO):
                xt = xt_cur
                if m + 1 < NMACRO:
                    xt_next = xin.tile([128, NBLK, MACRO], bf16)
                    nc.sync.dma_start(xt_next[:, 0:3, :], x_d[m + 1, :, 0:3, :])
                    nc.sync.dma_start(xt_next[:, 3:5, :], x_d[m + 1, :, 3:5, :])
                    xt_cur = xt_next

                # ping-pong: swap which buffer is pz0/pz1 each macro, so
                # trunk(m+1)'s first chain WARs against GELU1(m) (always
                # prompt: z-blk1(m) already needed it) instead of GELU0.
                ta = ps_z.tile([128, MACRO], f32, tag="pz")
                tb = ps_z.tile([128, MACRO], f32, tag="pz")
                pz0, pz1 = (ta, tb) if m % 2 == 0 else (tb, ta)
                zs = zsp.tile([128, 2, MACRO], bf16)

                if m == 0:
                    # col-interleaved so each x block is consumed as it
                    # lands; router(0) woven in once block 3 is here.
                    for b in range(3):
                        for pz, c in ((pz0, 0), (pz1, 128)):
                            nc.tensor.matmul(
                                pz, wt[:, b, c : c + 128], xt[:, b, :],
                                start=(b == 0), stop=False,
                            )
                    nc.tensor.matmul(
                        pz0, wt[:, 3, 0:128], xt[:, 3, :], start=False, stop=False
                    )
                    nc.tensor.matmul(
                        pz0, wt[:, 4, 0:128], xt[:, 4, :], start=False, stop=True
                    )
                    nc.scalar.activation(
                        out=zs[:, 0, :], in_=pz0, func=AF.Gelu, scale=1.0
                    )
                    emit_router(0, xt)
                    # softmax(0) emitted before GELU1 so tanh(0) runs on
                    # ACT as soon as the router matmuls finish
                    pw8_cur = emit_softmax(0)
                    nc.tensor.matmul(
                        pz1, wt[:, 3, 128:256], xt[:, 3, :], start=False, stop=False
                    )
                    nc.tensor.matmul(
                        pz1, wt[:, 4, 128:256], xt[:, 4, :], start=False, stop=True
                    )
                    nc.scalar.activation(
                        out=zs[:, 1, :], in_=pz1, func=AF.Gelu, scale=1.0
                    )
                else:
                    # pz0 chain first so GELU0 overlaps the pz1 chain;
                    # pwT transpose for this macro woven in early.
                    for b in range(NBLK):
                        nc.tensor.matmul(
                            pz0, wt[:, b, 0:128], xt[:, b, :],
                            start=(b == 0), stop=(b == 4),
                        )
                        if b == 1:
                            pwt_cur = emit_transpose(m, pw8_cur)
                    nc.scalar.activation(
                        out=zs[:, 0, :], in_=pz0, func=AF.Gelu, scale=1.0
                    )
                    for b in range(NBLK):
                        nc.tensor.matmul(
                            pz1, wt[:, b, 128:256], xt[:, b, :],
                            start=(b == 0), stop=(b == 4),
                        )
                    nc.scalar.activation(
                        out=zs[:, 1, :], in_=pz1, func=AF.Gelu, scale=1.0
                    )

                pe4 = ps_e.tile([128, SUB, 512], f32, tag="pe4")
                # z block 0 opens each bank's accumulation group
                for s in range(SUB):
                    nc.tensor.matmul(
                        pe4[:, s, 0:NCOL], zs[:, 0, ts(s, 128)], wh[:, 0, :],
                        start=True, stop=False,
                    )
                # router + softmax for the NEXT macro run here so pw8 is
                # always ready a full macro before its bias-dot.
                if m + 1 < NMACRO:
                    emit_router(m + 1, xt_cur)
                    pw8_next = emit_softmax(m + 1)
                else:
                    pw8_next = None
                if m == 0:
                    pwt_cur = emit_transpose(0, pw8_cur)
                for s in range(SUB):
                    nc.tensor.matmul(
                        pe4[:, s, 0:NCOL], zs[:, 1, ts(s, 128)], wh[:, 1, :],
                        start=False, stop=False,
                    )
                # bias-dot: out[tok, hk] = sum_p pw_p * be_p[hk] + bb[hk]
                pe4s = evacp.tile([128, SUB, BCOL], bf16)
                for s in range(SUB):
                    # K=64 from base {0,64}: subtile s slots live at
                    # partitions 32s..32s+8; the masked bias table zeroes
                    # every other contraction row.
                    q = 64 * (s // 2)
                    nc.tensor.matmul(
                        pe4[:, s, NCOL : NCOL + BCOL],
                        pwt_cur[q : q + 64, :],
                        bt[q : q + 64, s % 2, :],
                        start=False, stop=True, skip_group_check=True,
                    )
                # evac AFTER all bias-dots (an ACT read of pe4 between PE
                # writes to it serializes the PE queue): only the 40
                # bias-dot cols leave PSUM via ACT; the big products read
                # PSUM directly on DVE.
                nc.scalar.copy(
                    out=pe4s, in_=pe4[:, :, NCOL : NCOL + BCOL],
                )

                # combine: prod = pe4 * pw8 (DVE, PSUM read) -> bf16;
                # 2-level p-reduction: bf16 pair-add at DVE 2x, then f32
                # reduce of 4; Pool adds the bias-dot columns.
                osb = outp.tile([128, SUB, NHK], bf16, tag="osb")
                prod = prodp.tile([128, SUB, NHK, NP8], bf16, tag="prod")
                h4 = prodp.tile([128, SUB, NHK, 4], bf16, tag="h4")
                for p2 in range(2):
                    sl = slice(2 * p2, 2 * p2 + 2)
                    nc.vector.tensor_tensor(
                        out=prod[:, sl],
                        in0=pe4[:, sl, 0:NCOL].rearrange(
                            "p s (hk e) -> p s hk e", e=NP8
                        ),
                        in1=pw8_cur[:, sl, None, 0:NP8].to_broadcast(
                            [128, 2, NHK, NP8]
                        ),
                        op=ALU.mult,
                    )
                    # pair-add on Pool (reads SBUF bf16) to unload DVE
                    nc.gpsimd.tensor_tensor(
                        out=h4[:, sl], in0=prod[:, sl, :, 0:4],
                        in1=prod[:, sl, :, 4:8], op=ALU.add,
                    )
                    # bf16 out -> all-2-byte operands -> DVE 2x mode; the
                    # products/partials are already bf16 so the extra
                    # rounding is one step
                    with nc.allow_low_precision(reason="bf16 head combine"):
                        nc.vector.reduce_sum(
                            out=osb[:, sl], in_=h4[:, sl],
                            axis=mybir.AxisListType.X,
                        )
                osb2 = outp.tile([128, SUB, NHK], bf16, tag="osb2")
                nc.gpsimd.tensor_tensor(
                    out=osb2, in0=osb, in1=pe4s, op=ALU.add,
                )
                # output DMA on the same queue that produced osb2: FIFO
                # order means the issue never blocks another queue
                nc.gpsimd.dma_start(ov[m], osb2)

                pw8_cur = pw8_next

    nc.compile()
    return nc


def _host_weights(inp):
    """Rearrange weights into on-device layouts (host-side, one-time)."""
    import ml_dtypes

    bf16 = ml_dtypes.bfloat16
    f = np.float32

    # trunk weights; block 4 row 96 multiplies the ones row -> trunk bias
    wt = np.zeros((128, NBLK, TRUNK), f)
    for b in range(4):
        wt[:, b, :] = inp["trunk_w"][b * 128 : (b + 1) * 128]
    wt[:82, 4, :] = inp["trunk_w"][512:IN_DIM]
    wt[96, 4, :] = inp["trunk_b"]

    wr = np.zeros((128, 4, PHASES), f)
    for b in range(4):
        wr[:, b, :] = inp["router_w"][b * 128 : (b + 1) * 128]

    # z-matmul weights: col c = (h*20+k)*8 + p; p<7 experts, p=7 base
    wh_full = np.zeros((TRUNK, NHK, NP8), f)
    wh_full[:, :BINS, :PHASES] = np.transpose(inp["inst_exp_w"], (1, 2, 0))
    wh_full[:, BINS:, :PHASES] = np.transpose(inp["group_exp_w"], (1, 2, 0))
    wh_full[:, :BINS, PHASES] = inp["inst_base_w"]
    wh_full[:, BINS:, PHASES] = inp["group_base_w"]
    wh = (
        wh_full.reshape(TRUNK, NCOL).reshape(2, 128, NCOL).transpose(1, 0, 2)
    ).astype(f)

    # masked bias tables for the K=64 bias-dot: table s%2 has dr at
    # partitions 32s..32s+8 and zeros elsewhere
    dr = np.zeros((NP8, NHK), f)
    dr[:PHASES, :BINS] = inp["inst_exp_b"]
    dr[:PHASES, BINS:] = inp["group_exp_b"]
    dr[PHASES, :BINS] = inp["inst_base_b"]
    dr[PHASES, BINS:] = inp["group_base_b"]
    bt = np.zeros((128, 2, BCOL), f)
    for s in range(4):
        bt[32 * s : 32 * s + NP8, s % 2, :] = dr

    invc = np.tile(np.exp(-inp["router_b"])[None, :], (128, 1)).astype(f)
    ident = np.eye(128, dtype=f)

    return (
        wt.astype(bf16), wr.astype(bf16), wh.astype(bf16), bt.astype(bf16),
        invc, ident.astype(bf16),
    )


def kernel(**inputs):
    global LAST_RESULT
    import sys

    if "/opt/trn_rl_repo" not in sys.path:
        sys.path.insert(0, "/opt/trn_rl_repo")
    import ml_dtypes
    from concourse.bass_utils import run_bass_kernel_spmd

    bf16 = ml_dtypes.bfloat16

    inp = {k: np.asarray(v, dtype=np.float32) for k, v in inputs.items()}

    if "nc" not in _BUILT:
        _BUILT["nc"] = _build_module()
    nc = _BUILT["nc"]

    wt, wr, wh, bt, invc, ident = _host_weights(inp)

    x_full = np.concatenate(
        [inp["h_t"], inp["a_t"], inp["d_t"], inp["age_embed"]], axis=-1
    ).astype(bf16)  # [B, T, 594]
    # feature-major transpose + zero-pad 594 -> 640 (5 blocks of 128):
    # xT[c, m, k, b, j] = x[c, m*512 + j, b*128 + k]; padded feature 608
    # (k=96 of block 4) is a constant 1.0 that carries the trunk bias.
    pad = np.zeros((NCORES, NMACRO, MACRO, NBLK * 128 - IN_DIM), bf16)
    pad[:, :, :, 608 - IN_DIM] = 1.0
    x_pad = np.concatenate(
        [x_full.reshape(NCORES, NMACRO, MACRO, IN_DIM), pad], axis=-1
    ).reshape(NCORES, NMACRO, MACRO, NBLK, 128)
    xT = np.ascontiguousarray(np.transpose(x_pad, (0, 1, 4, 3, 2)))

    in_maps = []
    for c in range(NCORES):
        in_maps.append(
            {
                "x": xT[c], "wt": wt, "wr": wr, "wh": wh, "bt": bt,
                "invc": invc, "ident": ident,
            }
        )

    res = run_bass_kernel_spmd(nc, in_maps, core_ids=list(range(NCORES)))
    LAST_RESULT = res

    inst = np.empty((B, T, BINS), np.float32)
    grp = np.empty((B, T, BINS), np.float32)
    for c in range(NCORES):
        o = res.results[c]["out"].astype(np.float32)
        inst[c * B_LOC : (c + 1) * B_LOC] = o[:, 0:BINS].reshape(B_LOC, T, BINS)
        grp[c * B_LOC : (c + 1) * B_LOC] = o[:, BINS:].reshape(B_LOC, T, BINS)
    return inst, grp
